# revision 25
# baseline (speedup 1.0000x reference)
"""Trainium2 Bass kernel for nn_CBL_1632087573343 (boundary context loss).

Data-parallel over batch: 8 images -> 8 NeuronCores, one image per core.

Per-core algorithm (reproduces reference._context_loss for one image):
  - er image stored as bf16 [c-chunk(128) x 2, 66*128] flat slabs,
    processed in 2 row-halves; a 1-element-shifted copy (xodd) keeps the
    DVE tensor_tensor multiplies 4B-aligned (2x perf mode) for odd offsets.
  - For each of 12 canonical shifts s (+- pairs folded via weight
    W_s = valid + valid(.+s)) plus the norm pass (s=0):
      DVE: prod_c = er_c * er_c_shifted   (bf16, flat offset dy*128+dx)
      PE:  channel reduction via matmuls with ONE-HOT-COLUMN stationaries:
           block b (512 pixels) uses stationary = window view of a
           [128, 128+NB] tile whose only nonzero column selects output
           partition b; all blocks accumulate into one PSUM [128, 512]
           bank, so 16 blocks x 2 chunks of a half land as rows 0..15.
           Moving operand is the product (N=512 @ 2.4 GHz, LDW hidden).
      ACT: copy psum[0:16, :] -> st [16, 512]
      DMA: fan st out to the dot field tile [y=128, 2|128|2] ([y, x]).
  - Pointwise on [128 y, 132] tiles: dy handled by DMA-shifted copies of
    rn/seg/valid (engines cannot start at partition 1/2), dx by free-dim
    offsets.  cos = dot*rn*rn_s, d = cos - (seg==seg_s), A += d*d*W_s.
  - Reduce A / valid / gt_b; assemble per-image (loss_num, include).
Host combines: loss = sum(loss_num) / max(sum(include), 1).
"""

import sys

sys.path.insert(0, "/opt/trn_rl_repo")

import numpy as np

import concourse.bass as bass
import concourse.tile as tile
from concourse import bacc, mybir

DT = mybir.dt
F32 = DT.float32
BF16 = DT.bfloat16
I32 = DT.int32
ALU = mybir.AluOpType
ACTF = mybir.ActivationFunctionType
AX = mybir.AxisListType

B, C, H, W = 8, 256, 128, 128
HH = 64                          # rows per half
SLAB_ROWS = HH + 2               # rows resident per half (dy<=2 read-ahead)
L_SLAB = 8512                    # >= 66*128+4, padded to a 128B multiple
L_RED = HH * W                   # 8192 columns reduced per (half, shift)
NB = 16                          # 512-pixel blocks per (half, shift)
FX = 192                         # field tile free size (128B-aligned): 2 | 128 x | pad
FOFF = 2                         # x offset inside field tiles

# canonical half of the 24-shift set; even-dx first so the odd-dx slab copy
# (single-buffered) can load while even shifts compute
SHIFTS = [(0, 2), (1, -2), (1, 0), (1, 2), (2, -2), (2, 0), (2, 2),
          (0, 1), (1, -1), (1, 1), (2, -1), (2, 1)]


def _ap(t, offset, dims):
    return bass.AP(t.tensor, offset, [list(d) for d in dims])


def build_kernel(nc):
    er_d = nc.dram_tensor("er", [C, H, W], F32, kind="ExternalInput")
    seg_d = nc.dram_tensor("seg", [H, W], I32, kind="ExternalInput")
    gtb_d = nc.dram_tensor("gtb", [H, W], I32, kind="ExternalInput")
    out_d = nc.dram_tensor("out", [1, 2], F32, kind="ExternalOutput")

    with tile.TileContext(nc) as tc:
        _build(tc, er_d, seg_d, gtb_d, out_d)
    nc.compile()
    return nc


def _build(tc, er_d, seg_d, gtb_d, out_d):
    nc = tc.nc
    from contextlib import ExitStack

    with ExitStack() as ctx:
        const_p = ctx.enter_context(tc.tile_pool(name="const", bufs=1))
        er_p = ctx.enter_context(tc.tile_pool(name="erp", bufs=2))
        xo_p = ctx.enter_context(tc.tile_pool(name="xop", bufs=1))
        prod_p = ctx.enter_context(tc.tile_pool(name="prodp", bufs=1))
        field_p = ctx.enter_context(tc.tile_pool(name="fieldp", bufs=1))
        st_p = ctx.enter_context(tc.tile_pool(name="stp", bufs=2))
        scr_p = ctx.enter_context(tc.tile_pool(name="scrp", bufs=2))
        psum_p = ctx.enter_context(
            tc.tile_pool(name="psump", bufs=2, space="PSUM"))

        ones_f = const_p.tile([128, 32], F32, name="ones_f", tag="ones_f")
        nc.vector.memset(ones_f[:], 1.0)
        # one-hot column bank: sel[:, 128+NB-1-b : 256+NB-1-b] has its only
        # nonzero (ones) column at position b
        SELW = 320
        sel = const_p.tile([128, SELW], BF16, name="sel", tag="sel")
        nc.vector.memset(sel[:], 0.0)
        nc.vector.memset(sel[:, 128 + NB - 1:128 + NB], 1.0)

        P0 = 128 + NB - 1   # absolute position of the ones column

        def sel_view(b):
            # b+1 columns ending at the ones column: output rows 0..b,
            # row b = column sums. Short stationary keeps LDWEIGHTS tiny.
            return sel[:, P0 - b:P0 + 1]

        # ---- label fields ([y, x] layout) ------------------------------
        segi = field_p.tile([H, FX], I32, name="segi", tag="segi")
        nc.vector.memset(segi[:], 0)
        nc.sync.dma_start(out=segi[:, FOFF:FOFF + W], in_=seg_d.ap())
        gtbi = field_p.tile([H, FX], I32, name="gtbi", tag="gtbi")
        nc.vector.memset(gtbi[:], 0)
        nc.sync.dma_start(out=gtbi[:, FOFF:FOFF + W], in_=gtb_d.ap())

        segb = scr_p.tile([H, FX], BF16, name="segb", tag="segb")
        nc.vector.tensor_copy(segb[:], segi[:])
        gtbb = scr_p.tile([H, FX], BF16, name="gtbb", tag="gtbb")
        nc.vector.tensor_copy(gtbb[:], gtbi[:])
        gt_b = field_p.tile([H, FX], BF16, name="gt_b", tag="gt_b")
        nc.vector.tensor_tensor(gt_b[:], segb[:], gtbb[:], op=ALU.mult)

        # interior: x (free col) in [FOFF+2, FOFF+126), y (part) in [2,126)
        iox = scr_p.tile([H, FX], I32, name="iox", tag="iox")
        nc.gpsimd.iota(iox[:], [[1, FX]], channel_multiplier=0)
        xm0 = scr_p.tile([H, FX], BF16, name="xm0", tag="xm0")
        nc.vector.tensor_scalar(xm0[:], iox[:], FOFF + 2, None, op0=ALU.is_ge)
        xm1 = scr_p.tile([H, FX], BF16, name="xm1", tag="xm1")
        nc.vector.tensor_scalar(xm1[:], iox[:], FOFF + 126, None,
                                op0=ALU.is_lt)
        ioy = scr_p.tile([H, 32], I32, name="ioy", tag="ioy")
        nc.gpsimd.iota(ioy[:, 0:1], [[1, 1]], channel_multiplier=1)
        ym0 = scr_p.tile([H, 32], F32, name="ym0", tag="ym0")
        nc.vector.tensor_scalar(ym0[:, 0:1], ioy[:, 0:1], 2, None, op0=ALU.is_ge)
        ym1 = scr_p.tile([H, 32], F32, name="ym1", tag="ym1")
        nc.vector.tensor_scalar(ym1[:, 0:1], ioy[:, 0:1], 126, None, op0=ALU.is_lt)
        ym = scr_p.tile([H, 32], F32, name="ym", tag="ym")
        nc.vector.tensor_tensor(ym[:, 0:1], ym0[:, 0:1], ym1[:, 0:1], op=ALU.mult)

        valid = field_p.tile([H, FX], BF16, name="valid", tag="valid")
        nc.vector.tensor_tensor(valid[:], gt_b[:], xm0[:], op=ALU.mult)
        nc.vector.tensor_tensor(valid[:], valid[:], xm1[:], op=ALU.mult)
        nc.vector.tensor_scalar(valid[:], valid[:], ym[:, 0:1], None, op0=ALU.mult)

        # ---- dot fields ([y, x]) ---------------------------------------
        fields = {}
        for s in [(0, 0)] + SHIFTS:
            f = field_p.tile([H, FX], F32, name=f"dot_{s[0]}_{s[1]}",
                             tag=f"dot_{s[0]}_{s[1]}")
            nc.vector.memset(f[:], 0.0)
            fields[s] = f

        A = field_p.tile([H, FX], F32, name="accA", tag="accA")
        nc.vector.memset(A[:], 0.0)

        # ---- main per-half loop ----------------------------------------
        for h in range(2):
            r0 = HH * h
            nflat = (SLAB_ROWS if h == 0 else HH) * W
            er_ch, xo_ch = [], []
            for c in range(2):
                e = er_p.tile([128, L_SLAB], BF16, name=f"er{c}",
                              tag=f"er{c}")
                x = xo_p.tile([128, L_SLAB], BF16, name=f"xo{c}",
                              tag=f"xo{c}")
                # only the unloaded tails need zeroing
                nc.vector.memset(e[:, nflat:L_SLAB], 0.0)
                nc.gpsimd.dma_start(
                    out=_ap(e, 0, [[L_SLAB, 128], [1, nflat]]),
                    in_=_ap(er_d.ap(), c * 128 * H * W + r0 * W,
                            [[H * W, 128], [1, nflat]]))
                nodd = min(nflat, H * W - r0 * W - 1)
                nc.vector.memset(x[:, nodd:L_SLAB], 0.0)
                nc.gpsimd.dma_start(
                    out=_ap(x, 0, [[L_SLAB, 128], [1, nodd]]),
                    in_=_ap(er_d.ap(), c * 128 * H * W + r0 * W + 1,
                            [[H * W, 128], [1, nodd]]))
                er_ch.append(e)
                xo_ch.append(x)

            for s in [(0, 0)] + SHIFTS:
                dy, dx = s
                off = dy * W + dx
                prods = []
                for c in range(2):
                    p = prod_p.tile([128, L_RED], BF16, name=f"prod{c}",
                                    tag=f"prod{c}")
                    if dx % 2 == 0:
                        in1 = er_ch[c][:, off:off + L_RED]
                    else:
                        in1 = xo_ch[c][:, off - 1:off - 1 + L_RED]
                    nc.vector.tensor_tensor(
                        p[:], er_ch[c][:, 0:L_RED], in1, op=ALU.mult)
                    prods.append(p)

                # block b -> psum row b (one-hot stationary); the block's
                # 512 pixels are the strided y-rows {b, b+16, b+32, b+48}
                # so the staging tile fans out with canonical DMAs below.
                ps = psum_p.tile([128, 512], F32, name="ps", tag="ps")
                n_mm = 2 * NB
                j = 0
                # descending b: the first matmul (b=NB-1) covers rows
                # [0:NB] and start=True-initializes them; later partial
                # writes accumulate into initialized rows only.
                for c in range(2):
                    for b in reversed(range(NB)):
                        nc.tensor.matmul(
                            ps[0:b + 1, 0:512], sel_view(b),
                            _ap(prods[c], 128 * b,
                                [[L_RED, 128], [128 * NB, 4], [1, W]]),
                            start=(j == 0), stop=(j == n_mm - 1),
                            skip_group_check=True)
                        j += 1

                st = st_p.tile([NB, 512], F32, name="st", tag="st")
                nc.scalar.copy(st[:], ps[0:NB, 0:512])

                # st[g, 128q + x] = dot(y = 16q + g, x): 4 DMAs, each to
                # 16 contiguous field partitions (pure partition dim0)
                f = fields[s]
                for q in range(4):
                    nc.sync.dma_start(
                        out=_ap(f, (r0 + 16 * q) * FX + FOFF,
                                [[FX, NB], [1, W]]),
                        in_=_ap(st, 128 * q, [[512, NB], [1, W]]))

        # ---- rn = 1 / max(sqrt(n2), eps) -------------------------------
        rn1 = scr_p.tile([H, FX], F32, name="rn1", tag="rn1")
        nc.vector.memset(rn1[:], 0.0)
        nc.scalar.activation(rn1[:], fields[(0, 0)][:], ACTF.Sqrt)
        nc.vector.tensor_scalar(rn1[:], rn1[:], 1e-8, None, op0=ALU.max)
        rn = field_p.tile([H, FX], F32, name="rn", tag="rn")
        nc.vector.reciprocal(rn[:], rn1[:])

        # ---- dy-shifted copies (engines can't start at partition k) ----
        # f_dk[y, x] = f[y + k, x]; tail rows zero.
        shifted = {0: {"rn": rn, "segi": segi, "valid": valid}}
        for k in (1, 2):
            sd = {}
            for nm, src in (("rn", rn), ("segi", segi), ("valid", valid)):
                t = field_p.tile([H, FX], src.dtype, name=f"{nm}_d{k}",
                                 tag=f"{nm}_d{k}")
                nc.vector.memset(t[:], 0)
                nc.sync.dma_start(
                    out=_ap(t, 0, [[FX, H - k], [1, FX]]),
                    in_=_ap(src, k * FX, [[FX, H - k], [1, FX]]))
                sd[nm] = t
            shifted[k] = sd

        # ---- pointwise per shift ---------------------------------------
        for s in SHIFTS:
            dy, dx = s
            b_ = np.s_[:, FOFF:FOFF + W]
            sh = np.s_[:, FOFF + dx:FOFF + dx + W]
            rn_s = shifted[dy]["rn"]
            segi_s = shifted[dy]["segi"]
            valid_s = shifted[dy]["valid"]

            lab = scr_p.tile([H, FX], BF16, name="lab", tag="lab")
            nc.vector.tensor_tensor(lab[b_], segi[b_], segi_s[sh],
                                    op=ALU.is_equal)
            Wt = scr_p.tile([H, FX], BF16, name="Wt", tag="Wt")
            nc.vector.tensor_tensor(Wt[b_], valid[b_], valid_s[sh],
                                    op=ALU.add)
            t1 = scr_p.tile([H, FX], F32, name="t1", tag="t1")
            nc.vector.tensor_tensor(t1[b_], fields[s][b_], rn[b_],
                                    op=ALU.mult)
            cosb = scr_p.tile([H, FX], BF16, name="cosb", tag="cosb")
            nc.vector.tensor_tensor(cosb[b_], t1[b_], rn_s[sh], op=ALU.mult)
            d = scr_p.tile([H, FX], BF16, name="d", tag="d")
            nc.vector.tensor_tensor(d[b_], cosb[b_], lab[b_],
                                    op=ALU.subtract)
            e2 = scr_p.tile([H, FX], BF16, name="e2", tag="e2")
            nc.vector.tensor_tensor(e2[b_], d[b_], d[b_], op=ALU.mult)
            fw = scr_p.tile([H, FX], BF16, name="fw", tag="fw")
            nc.vector.tensor_tensor(fw[b_], e2[b_], Wt[b_], op=ALU.mult)
            nc.vector.tensor_tensor(A[b_], A[b_], fw[b_], op=ALU.add)

        # ---- final reduction -------------------------------------------
        R = scr_p.tile([128, 32], F32, name="R", tag="R")
        nc.vector.memset(R[:], 0.0)
        nc.vector.tensor_reduce(R[:, 0:1], A[:], axis=AX.X, op=ALU.add)
        nc.vector.tensor_reduce(R[:, 1:2], valid[:], axis=AX.X, op=ALU.add)
        nc.vector.tensor_reduce(R[:, 2:3], gt_b[:], axis=AX.X, op=ALU.add)

        ps2 = psum_p.tile([128, 512], F32, name="ps2", tag="ps")
        nc.tensor.matmul(ps2[0:1, 0:4], ones_f[:, 0:1], R[:, 0:4],
                         start=True, stop=True)
        scal = scr_p.tile([1, 32], F32, name="scal", tag="scal")
        nc.scalar.copy(scal[0:1, 0:4], ps2[0:1, 0:4])
        # scal: 0=S, 1=cnt, 2=gtbsum | 4=include, 5=max(cnt,1), 6=1/max, 7=loss
        nc.vector.tensor_scalar(scal[0:1, 4:5], scal[0:1, 2:3], 0.0, None,
                                op0=ALU.is_gt)
        nc.vector.tensor_scalar(scal[0:1, 5:6], scal[0:1, 1:2], 1.0, None,
                                op0=ALU.max)
        nc.vector.reciprocal(scal[0:1, 6:7], scal[0:1, 5:6])
        nc.vector.tensor_tensor(scal[0:1, 7:8], scal[0:1, 0:1],
                                scal[0:1, 6:7], op=ALU.mult)
        nc.vector.tensor_tensor(scal[0:1, 7:8], scal[0:1, 7:8],
                                scal[0:1, 4:5], op=ALU.mult)
        nc.vector.tensor_scalar(scal[0:1, 7:8], scal[0:1, 7:8],
                                1.0 / 24.0, None, op0=ALU.mult)

        outt = scr_p.tile([1, 32], F32, name="outt", tag="outt")
        nc.vector.tensor_copy(outt[0:1, 0:1], scal[0:1, 7:8])
        nc.vector.tensor_copy(outt[0:1, 1:2], scal[0:1, 4:5])
        nc.sync.dma_start(out=out_d.ap(), in_=outt[0:1, 0:2])


_NC_CACHE = {}


def get_nc():
    if "nc" not in _NC_CACHE:
        nc = bacc.Bacc("TRN2", target_bir_lowering=False, debug=False)
        build_kernel(nc)
        _NC_CACHE["nc"] = nc
    return _NC_CACHE["nc"]


def kernel(er_input, seg_label, gt_boundary_seg):
    er = np.ascontiguousarray(np.asarray(er_input, dtype=np.float32))
    seg = np.ascontiguousarray(np.asarray(seg_label, dtype=np.int32))
    gtb = np.ascontiguousarray(np.asarray(gt_boundary_seg, dtype=np.int32))
    assert er.shape == (B, C, H, W), er.shape

    nc = get_nc()
    from concourse.bass_utils import run_bass_kernel_spmd

    in_maps = [
        {"er": er[i], "seg": seg[i], "gtb": gtb[i]} for i in range(B)
    ]
    res = run_bass_kernel_spmd(nc, in_maps, list(range(B)))
    outs = [res.results[i]["out"] for i in range(B)]
    loss_nums = np.array([o[0, 0] for o in outs], dtype=np.float64)
    incs = np.array([o[0, 1] for o in outs], dtype=np.float64)
    loss = loss_nums.sum() / max(incs.sum(), 1.0)
    return np.float32(loss)


# revision 28
# speedup vs baseline: 1.2747x; 1.2747x over previous
"""Trainium2 Bass kernel for nn_CBL_1632087573343 (boundary context loss).

Data-parallel over batch: 8 images -> 8 NeuronCores, one image per core.

Per-core algorithm (reproduces reference._context_loss for one image):
  - er image stored as bf16 [c-chunk(128) x 2, 66*128] flat slabs,
    processed in 2 row-halves; a 1-element-shifted copy (xodd) keeps the
    DVE tensor_tensor multiplies 4B-aligned (2x perf mode) for odd offsets.
  - For each of 12 canonical shifts s (+- pairs folded via weight
    W_s = valid + valid(.+s)) plus the norm pass (s=0):
      DVE: prod_c = er_c * er_c_shifted   (bf16, flat offset dy*128+dx)
      PE:  channel reduction via matmuls with ONE-HOT-COLUMN stationaries:
           block b (512 pixels) uses stationary = window view of a
           [128, 128+NB] tile whose only nonzero column selects output
           partition b; all blocks accumulate into one PSUM [128, 512]
           bank, so 16 blocks x 2 chunks of a half land as rows 0..15.
           Moving operand is the product (N=512 @ 2.4 GHz, LDW hidden).
      ACT: copy psum[0:16, :] -> st [16, 512]
      DMA: fan st out to the dot field tile [y=128, 2|128|2] ([y, x]).
  - Pointwise on [128 y, 132] tiles: dy handled by DMA-shifted copies of
    rn/seg/valid (engines cannot start at partition 1/2), dx by free-dim
    offsets.  cos = dot*rn*rn_s, d = cos - (seg==seg_s), A += d*d*W_s.
  - Reduce A / valid / gt_b; assemble per-image (loss_num, include).
Host combines: loss = sum(loss_num) / max(sum(include), 1).
"""

import sys

sys.path.insert(0, "/opt/trn_rl_repo")

import numpy as np

import concourse.bass as bass
import concourse.tile as tile
from concourse import bacc, mybir

DT = mybir.dt
F32 = DT.float32
BF16 = DT.bfloat16
I32 = DT.int32
ALU = mybir.AluOpType
ACTF = mybir.ActivationFunctionType
AX = mybir.AxisListType

B, C, H, W = 8, 256, 128, 128
HH = 64                          # rows per half
SLAB_ROWS = HH + 2               # rows resident per half (dy<=2 read-ahead)
L_SLAB = 8512                    # >= 66*128+4, padded to a 128B multiple
L_RED = HH * W                   # 8192 columns reduced per (half, shift)
NB = 16                          # 512-pixel blocks per (half, shift)
FX = 192                         # field tile free size (128B-aligned): 2 | 128 x | pad
FOFF = 2                         # x offset inside field tiles

# canonical half of the 24-shift set; even-dx first so the odd-dx slab copy
# (single-buffered) can load while even shifts compute
SHIFTS = [(0, 2), (1, -2), (1, 0), (1, 2), (2, -2), (2, 0), (2, 2),
          (0, 1), (1, -1), (1, 1), (2, -1), (2, 1)]


def _ap(t, offset, dims):
    return bass.AP(t.tensor, offset, [list(d) for d in dims])


def build_kernel(nc):
    er_d = nc.dram_tensor("er", [C, H, W], F32, kind="ExternalInput")
    seg_d = nc.dram_tensor("seg", [H, W], I32, kind="ExternalInput")
    gtb_d = nc.dram_tensor("gtb", [H, W], I32, kind="ExternalInput")
    out_d = nc.dram_tensor("out", [1, 2], F32, kind="ExternalOutput")

    with tile.TileContext(nc) as tc:
        _build(tc, er_d, seg_d, gtb_d, out_d)
    nc.compile()
    return nc


def _build(tc, er_d, seg_d, gtb_d, out_d):
    nc = tc.nc
    from contextlib import ExitStack

    with ExitStack() as ctx:
        const_p = ctx.enter_context(tc.tile_pool(name="const", bufs=1))
        er_p = ctx.enter_context(tc.tile_pool(name="erp", bufs=2))
        xo_p = ctx.enter_context(tc.tile_pool(name="xop", bufs=1))
        prod_p = ctx.enter_context(tc.tile_pool(name="prodp", bufs=2))
        field_p = ctx.enter_context(tc.tile_pool(name="fieldp", bufs=1))
        st_p = ctx.enter_context(tc.tile_pool(name="stp", bufs=3))
        scr_p = ctx.enter_context(tc.tile_pool(name="scrp", bufs=1))
        psum_p = ctx.enter_context(
            tc.tile_pool(name="psump", bufs=3, space="PSUM"))

        ones_f = const_p.tile([128, 32], F32, name="ones_f", tag="ones_f")
        nc.vector.memset(ones_f[:], 1.0)
        # one-hot column bank: sel[:, 128+NB-1-b : 256+NB-1-b] has its only
        # nonzero (ones) column at position b
        SELW = 320
        sel = const_p.tile([128, SELW], BF16, name="sel", tag="sel")
        nc.gpsimd.memset(sel[:], 0.0)
        nc.vector.memset(sel[:, 128 + NB - 1:128 + NB], 1.0)

        P0 = 128 + NB - 1   # absolute position of the ones column

        def sel_view(b):
            # b+1 columns ending at the ones column: output rows 0..b,
            # row b = column sums. Short stationary keeps LDWEIGHTS tiny.
            return sel[:, P0 - b:P0 + 1]

        # ---- label fields ([y, x] layout) ------------------------------
        segi = field_p.tile([H, FX], I32, name="segi", tag="segi")
        nc.gpsimd.memset(segi[:], 0)
        nc.sync.dma_start(out=segi[:, FOFF:FOFF + W], in_=seg_d.ap())
        gtbi = field_p.tile([H, FX], I32, name="gtbi", tag="gtbi")
        nc.gpsimd.memset(gtbi[:], 0)
        nc.sync.dma_start(out=gtbi[:, FOFF:FOFF + W], in_=gtb_d.ap())

        segb = scr_p.tile([H, FX], BF16, name="segb", tag="segb")
        nc.vector.tensor_copy(segb[:], segi[:])
        gtbb = scr_p.tile([H, FX], BF16, name="gtbb", tag="gtbb")
        nc.vector.tensor_copy(gtbb[:], gtbi[:])
        gt_b = field_p.tile([H, FX], BF16, name="gt_b", tag="gt_b")
        nc.vector.tensor_tensor(gt_b[:], segb[:], gtbb[:], op=ALU.mult)

        # interior: x (free col) in [FOFF+2, FOFF+126), y (part) in [2,126)
        iox = scr_p.tile([H, FX], I32, name="iox", tag="iox")
        nc.gpsimd.iota(iox[:], [[1, FX]], channel_multiplier=0)
        xm0 = scr_p.tile([H, FX], BF16, name="xm0", tag="xm0")
        nc.vector.tensor_scalar(xm0[:], iox[:], FOFF + 2, None, op0=ALU.is_ge)
        xm1 = scr_p.tile([H, FX], BF16, name="xm1", tag="xm1")
        nc.vector.tensor_scalar(xm1[:], iox[:], FOFF + 126, None,
                                op0=ALU.is_lt)
        ioy = scr_p.tile([H, 32], I32, name="ioy", tag="ioy")
        nc.gpsimd.iota(ioy[:, 0:1], [[1, 1]], channel_multiplier=1)
        ym0 = scr_p.tile([H, 32], F32, name="ym0", tag="ym0")
        nc.vector.tensor_scalar(ym0[:, 0:1], ioy[:, 0:1], 2, None, op0=ALU.is_ge)
        ym1 = scr_p.tile([H, 32], F32, name="ym1", tag="ym1")
        nc.vector.tensor_scalar(ym1[:, 0:1], ioy[:, 0:1], 126, None, op0=ALU.is_lt)
        ym = scr_p.tile([H, 32], F32, name="ym", tag="ym")
        nc.vector.tensor_tensor(ym[:, 0:1], ym0[:, 0:1], ym1[:, 0:1], op=ALU.mult)

        valid = field_p.tile([H, FX], BF16, name="valid", tag="valid")
        nc.vector.tensor_tensor(valid[:], gt_b[:], xm0[:], op=ALU.mult)
        nc.vector.tensor_tensor(valid[:], valid[:], xm1[:], op=ALU.mult)
        nc.vector.tensor_scalar(valid[:], valid[:], ym[:, 0:1], None, op0=ALU.mult)

        # ---- dot fields ([y, x]) ---------------------------------------
        fields = {}
        for s in [(0, 0)] + SHIFTS:
            f = field_p.tile([H, FX], F32, name=f"dot_{s[0]}_{s[1]}",
                             tag=f"dot_{s[0]}_{s[1]}")
            nc.gpsimd.memset(f[:], 0.0)
            fields[s] = f

        A = field_p.tile([H, FX], F32, name="accA", tag="accA")
        nc.gpsimd.memset(A[:], 0.0)

        # ---- main per-half loop ----------------------------------------
        for h in range(2):
            r0 = HH * h
            nflat = (SLAB_ROWS if h == 0 else HH) * W
            er_ch, xo_ch = [], []
            for c in range(2):
                e = er_p.tile([128, L_SLAB], BF16, name=f"er{c}",
                              tag=f"er{c}")
                x = xo_p.tile([128, L_SLAB], BF16, name=f"xo{c}",
                              tag=f"xo{c}")
                # only the unloaded tails need zeroing
                nc.gpsimd.memset(e[:, nflat:L_SLAB], 0.0)
                nc.gpsimd.dma_start(
                    out=_ap(e, 0, [[L_SLAB, 128], [1, nflat]]),
                    in_=_ap(er_d.ap(), c * 128 * H * W + r0 * W,
                            [[H * W, 128], [1, nflat]]))
                nodd = min(nflat, H * W - r0 * W - 1)
                nc.gpsimd.memset(x[:, nodd:L_SLAB], 0.0)
                nc.gpsimd.dma_start(
                    out=_ap(x, 0, [[L_SLAB, 128], [1, nodd]]),
                    in_=_ap(er_d.ap(), c * 128 * H * W + r0 * W + 1,
                            [[H * W, 128], [1, nodd]]))
                er_ch.append(e)
                xo_ch.append(x)

            for s in [(0, 0)] + SHIFTS:
                dy, dx = s
                off = dy * W + dx
                prods = []
                for c in range(2):
                    p = prod_p.tile([128, L_RED], BF16, name=f"prod{c}",
                                    tag=f"prod{c}")
                    if dx % 2 == 0:
                        in1 = er_ch[c][:, off:off + L_RED]
                    else:
                        in1 = xo_ch[c][:, off - 1:off - 1 + L_RED]
                    nc.vector.tensor_tensor(
                        p[:], er_ch[c][:, 0:L_RED], in1, op=ALU.mult)
                    prods.append(p)

                # block b -> psum row b (one-hot stationary); the block's
                # 512 pixels are the strided y-rows {b, b+16, b+32, b+48}
                # so the staging tile fans out with canonical DMAs below.
                ps = psum_p.tile([128, 512], F32, name="ps", tag="ps")
                n_mm = 2 * NB
                j = 0
                # descending b: the first matmul (b=NB-1) covers rows
                # [0:NB] and start=True-initializes them; later partial
                # writes accumulate into initialized rows only.
                for b in reversed(range(NB)):
                    for c in range(2):
                        nc.tensor.matmul(
                            ps[0:b + 1, 0:512], sel_view(b),
                            _ap(prods[c], 128 * b,
                                [[L_RED, 128], [128 * NB, 4], [1, W]]),
                            start=(j == 0), stop=(j == n_mm - 1),
                            skip_group_check=True)
                        j += 1

                st = st_p.tile([NB, 512], F32, name="st", tag="st")
                nc.scalar.copy(st[:], ps[0:NB, 0:512])

                # st[g, 128q + x] = dot(y = 16q + g, x): 4 DMAs, each to
                # 16 contiguous field partitions (pure partition dim0)
                f = fields[s]
                for q in range(4):
                    nc.sync.dma_start(
                        out=_ap(f, (r0 + 16 * q) * FX + FOFF,
                                [[FX, NB], [1, W]]),
                        in_=_ap(st, 128 * q, [[512, NB], [1, W]]))

        # ---- rn = 1 / max(sqrt(n2), eps) -------------------------------
        rn1 = scr_p.tile([H, FX], F32, name="rn1", tag="rn1")
        nc.vector.memset(rn1[:], 0.0)
        nc.scalar.activation(rn1[:], fields[(0, 0)][:], ACTF.Sqrt)
        nc.vector.tensor_scalar(rn1[:], rn1[:], 1e-8, None, op0=ALU.max)
        rn = field_p.tile([H, FX], F32, name="rn", tag="rn")
        nc.vector.reciprocal(rn[:], rn1[:])

        # ---- dy-shifted copies (engines can't start at partition k) ----
        # f_dk[y, x] = f[y + k, x]; tail rows zero.
        shifted = {0: {"rn": rn, "segi": segi, "valid": valid}}
        for k in (1, 2):
            sd = {}
            for nm, src in (("rn", rn), ("segi", segi), ("valid", valid)):
                t = field_p.tile([H, FX], src.dtype, name=f"{nm}_d{k}",
                                 tag=f"{nm}_d{k}")
                nc.gpsimd.memset(t[:], 0)
                nc.sync.dma_start(
                    out=_ap(t, 0, [[FX, H - k], [1, FX]]),
                    in_=_ap(src, k * FX, [[FX, H - k], [1, FX]]))
                sd[nm] = t
            shifted[k] = sd

        # ---- pointwise per shift ---------------------------------------
        for s in SHIFTS:
            dy, dx = s
            b_ = np.s_[:, FOFF:FOFF + W]
            sh = np.s_[:, FOFF + dx:FOFF + dx + W]
            rn_s = shifted[dy]["rn"]
            segi_s = shifted[dy]["segi"]
            valid_s = shifted[dy]["valid"]

            lab = scr_p.tile([H, FX], BF16, name="lab", tag="lab")
            nc.vector.tensor_tensor(lab[b_], segi[b_], segi_s[sh],
                                    op=ALU.is_equal)
            Wt = scr_p.tile([H, FX], BF16, name="Wt", tag="Wt")
            nc.vector.tensor_tensor(Wt[b_], valid[b_], valid_s[sh],
                                    op=ALU.add)
            t1 = scr_p.tile([H, FX], F32, name="t1", tag="t1")
            nc.vector.tensor_tensor(t1[b_], fields[s][b_], rn[b_],
                                    op=ALU.mult)
            cosb = scr_p.tile([H, FX], BF16, name="cosb", tag="cosb")
            nc.vector.tensor_tensor(cosb[b_], t1[b_], rn_s[sh], op=ALU.mult)
            d = scr_p.tile([H, FX], BF16, name="d", tag="d")
            nc.vector.tensor_tensor(d[b_], cosb[b_], lab[b_],
                                    op=ALU.subtract)
            e2 = scr_p.tile([H, FX], BF16, name="e2", tag="e2")
            nc.vector.tensor_tensor(e2[b_], d[b_], d[b_], op=ALU.mult)
            fw = scr_p.tile([H, FX], BF16, name="fw", tag="fw")
            nc.vector.tensor_tensor(fw[b_], e2[b_], Wt[b_], op=ALU.mult)
            nc.vector.tensor_tensor(A[b_], A[b_], fw[b_], op=ALU.add)

        # ---- final reduction -------------------------------------------
        R = scr_p.tile([128, 32], F32, name="R", tag="R")
        nc.vector.memset(R[:], 0.0)
        nc.vector.tensor_reduce(R[:, 0:1], A[:], axis=AX.X, op=ALU.add)
        nc.vector.tensor_reduce(R[:, 1:2], valid[:], axis=AX.X, op=ALU.add)
        nc.vector.tensor_reduce(R[:, 2:3], gt_b[:], axis=AX.X, op=ALU.add)

        ps2 = psum_p.tile([128, 512], F32, name="ps2", tag="ps")
        nc.tensor.matmul(ps2[0:1, 0:4], ones_f[:, 0:1], R[:, 0:4],
                         start=True, stop=True)
        scal = scr_p.tile([1, 32], F32, name="scal", tag="scal")
        nc.scalar.copy(scal[0:1, 0:4], ps2[0:1, 0:4])
        # scal: 0=S, 1=cnt, 2=gtbsum | 4=include, 5=max(cnt,1), 6=1/max, 7=loss
        nc.vector.tensor_scalar(scal[0:1, 4:5], scal[0:1, 2:3], 0.0, None,
                                op0=ALU.is_gt)
        nc.vector.tensor_scalar(scal[0:1, 5:6], scal[0:1, 1:2], 1.0, None,
                                op0=ALU.max)
        nc.vector.reciprocal(scal[0:1, 6:7], scal[0:1, 5:6])
        nc.vector.tensor_tensor(scal[0:1, 7:8], scal[0:1, 0:1],
                                scal[0:1, 6:7], op=ALU.mult)
        nc.vector.tensor_tensor(scal[0:1, 7:8], scal[0:1, 7:8],
                                scal[0:1, 4:5], op=ALU.mult)
        nc.vector.tensor_scalar(scal[0:1, 7:8], scal[0:1, 7:8],
                                1.0 / 24.0, None, op0=ALU.mult)

        outt = scr_p.tile([1, 32], F32, name="outt", tag="outt")
        nc.vector.tensor_copy(outt[0:1, 0:1], scal[0:1, 7:8])
        nc.vector.tensor_copy(outt[0:1, 1:2], scal[0:1, 4:5])
        nc.sync.dma_start(out=out_d.ap(), in_=outt[0:1, 0:2])


_NC_CACHE = {}


def get_nc():
    if "nc" not in _NC_CACHE:
        nc = bacc.Bacc("TRN2", target_bir_lowering=False, debug=False)
        build_kernel(nc)
        _NC_CACHE["nc"] = nc
    return _NC_CACHE["nc"]


def kernel(er_input, seg_label, gt_boundary_seg):
    er = np.ascontiguousarray(np.asarray(er_input, dtype=np.float32))
    seg = np.ascontiguousarray(np.asarray(seg_label, dtype=np.int32))
    gtb = np.ascontiguousarray(np.asarray(gt_boundary_seg, dtype=np.int32))
    assert er.shape == (B, C, H, W), er.shape

    nc = get_nc()
    from concourse.bass_utils import run_bass_kernel_spmd

    in_maps = [
        {"er": er[i], "seg": seg[i], "gtb": gtb[i]} for i in range(B)
    ]
    res = run_bass_kernel_spmd(nc, in_maps, list(range(B)))
    outs = [res.results[i]["out"] for i in range(B)]
    loss_nums = np.array([o[0, 0] for o in outs], dtype=np.float64)
    incs = np.array([o[0, 1] for o in outs], dtype=np.float64)
    loss = loss_nums.sum() / max(incs.sum(), 1.0)
    return np.float32(loss)


# revision 30
# speedup vs baseline: 1.3426x; 1.0533x over previous
"""Trainium2 Bass kernel for nn_CBL_1632087573343 (boundary context loss).

Data-parallel over batch: 8 images -> 8 NeuronCores, one image per core.

Per-core algorithm (reproduces reference._context_loss for one image):
  - er image stored as bf16 [c-chunk(128) x 2, 66*128] flat slabs,
    processed in 2 row-halves; a 1-element-shifted copy (xodd) keeps the
    DVE tensor_tensor multiplies 4B-aligned (2x perf mode) for odd offsets.
  - For each of 12 canonical shifts s (+- pairs folded via weight
    W_s = valid + valid(.+s)) plus the norm pass (s=0):
      DVE: prod_c = er_c * er_c_shifted   (bf16, flat offset dy*128+dx)
      PE:  channel reduction via matmuls with ONE-HOT-COLUMN stationaries:
           block b (512 pixels) uses stationary = window view of a
           [128, 128+NB] tile whose only nonzero column selects output
           partition b; all blocks accumulate into one PSUM [128, 512]
           bank, so 16 blocks x 2 chunks of a half land as rows 0..15.
           Moving operand is the product (N=512 @ 2.4 GHz, LDW hidden).
      ACT: copy psum[0:16, :] -> st [16, 512]
      DMA: fan st out to the dot field tile [y=128, 2|128|2] ([y, x]).
  - Pointwise on [128 y, 132] tiles: dy handled by DMA-shifted copies of
    rn/seg/valid (engines cannot start at partition 1/2), dx by free-dim
    offsets.  cos = dot*rn*rn_s, d = cos - (seg==seg_s), A += d*d*W_s.
  - Reduce A / valid / gt_b; assemble per-image (loss_num, include).
Host combines: loss = sum(loss_num) / max(sum(include), 1).
"""

import sys

sys.path.insert(0, "/opt/trn_rl_repo")

import numpy as np

import concourse.bass as bass
import concourse.tile as tile
from concourse import bacc, mybir

DT = mybir.dt
F32 = DT.float32
BF16 = DT.bfloat16
I32 = DT.int32
ALU = mybir.AluOpType
ACTF = mybir.ActivationFunctionType
AX = mybir.AxisListType

B, C, H, W = 8, 256, 128, 128
HH = 64                          # rows per half
SLAB_ROWS = HH + 2               # rows resident per half (dy<=2 read-ahead)
L_SLAB = 8512                    # >= 66*128+4, padded to a 128B multiple
L_RED = HH * W                   # 8192 columns reduced per (half, shift)
NB = 16                          # 512-pixel blocks per (half, shift)
FX = 192                         # field tile free size (128B-aligned): 2 | 128 x | pad
FOFF = 2                         # x offset inside field tiles

# canonical half of the 24-shift set; even-dx first so the odd-dx slab copy
# (single-buffered) can load while even shifts compute
SHIFTS = [(0, 2), (1, -2), (1, 0), (1, 2), (2, -2), (2, 0), (2, 2),
          (0, 1), (1, -1), (1, 1), (2, -1), (2, 1)]


def _ap(t, offset, dims):
    return bass.AP(t.tensor, offset, [list(d) for d in dims])


def build_kernel(nc):
    er_d = nc.dram_tensor("er", [C, H, W], F32, kind="ExternalInput")
    seg_d = nc.dram_tensor("seg", [H, W], I32, kind="ExternalInput")
    gtb_d = nc.dram_tensor("gtb", [H, W], I32, kind="ExternalInput")
    out_d = nc.dram_tensor("out", [1, 2], F32, kind="ExternalOutput")

    with tile.TileContext(nc) as tc:
        _build(tc, er_d, seg_d, gtb_d, out_d)
    nc.compile()
    return nc


def _build(tc, er_d, seg_d, gtb_d, out_d):
    nc = tc.nc
    from contextlib import ExitStack

    with ExitStack() as ctx:
        const_p = ctx.enter_context(tc.tile_pool(name="const", bufs=1))
        er_p = ctx.enter_context(tc.tile_pool(name="erp", bufs=2))
        xo_p = ctx.enter_context(tc.tile_pool(name="xop", bufs=1))
        prod_p = ctx.enter_context(tc.tile_pool(name="prodp", bufs=2))
        field_p = ctx.enter_context(tc.tile_pool(name="fieldp", bufs=1))
        st_p = ctx.enter_context(tc.tile_pool(name="stp", bufs=3))
        scr_p = ctx.enter_context(tc.tile_pool(name="scrp", bufs=1))
        psum_p = ctx.enter_context(
            tc.tile_pool(name="psump", bufs=3, space="PSUM"))

        ones_f = const_p.tile([128, 32], F32, name="ones_f", tag="ones_f")
        nc.vector.memset(ones_f[:], 1.0)
        # one-hot column bank: sel[:, 128+NB-1-b : 256+NB-1-b] has its only
        # nonzero (ones) column at position b
        SELW = 320
        sel = const_p.tile([128, SELW], BF16, name="sel", tag="sel")
        nc.gpsimd.memset(sel[:], 0.0)
        nc.vector.memset(sel[:, 128 + NB - 1:128 + NB], 1.0)

        P0 = 128 + NB - 1   # absolute position of the ones column

        def sel_view(b):
            # b+1 columns ending at the ones column: output rows 0..b,
            # row b = column sums. Short stationary keeps LDWEIGHTS tiny.
            return sel[:, P0 - b:P0 + 1]

        # ---- label fields ([y, x] layout) ------------------------------
        segi = field_p.tile([H, FX], I32, name="segi", tag="segi")
        nc.gpsimd.memset(segi[:], 0)
        nc.sync.dma_start(out=segi[:, FOFF:FOFF + W], in_=seg_d.ap())
        gtbi = field_p.tile([H, FX], I32, name="gtbi", tag="gtbi")
        nc.gpsimd.memset(gtbi[:], 0)
        nc.sync.dma_start(out=gtbi[:, FOFF:FOFF + W], in_=gtb_d.ap())

        segb = scr_p.tile([H, FX], BF16, name="segb", tag="segb")
        nc.vector.tensor_copy(segb[:], segi[:])
        gtbb = scr_p.tile([H, FX], BF16, name="gtbb", tag="gtbb")
        nc.vector.tensor_copy(gtbb[:], gtbi[:])
        gt_b = field_p.tile([H, FX], BF16, name="gt_b", tag="gt_b")
        nc.vector.tensor_tensor(gt_b[:], segb[:], gtbb[:], op=ALU.mult)

        # interior: x (free col) in [FOFF+2, FOFF+126), y (part) in [2,126)
        iox = scr_p.tile([H, FX], I32, name="iox", tag="iox")
        nc.gpsimd.iota(iox[:], [[1, FX]], channel_multiplier=0)
        xm0 = scr_p.tile([H, FX], BF16, name="xm0", tag="xm0")
        nc.vector.tensor_scalar(xm0[:], iox[:], FOFF + 2, None, op0=ALU.is_ge)
        xm1 = scr_p.tile([H, FX], BF16, name="xm1", tag="xm1")
        nc.vector.tensor_scalar(xm1[:], iox[:], FOFF + 126, None,
                                op0=ALU.is_lt)
        ioy = scr_p.tile([H, 32], I32, name="ioy", tag="ioy")
        nc.gpsimd.iota(ioy[:, 0:1], [[1, 1]], channel_multiplier=1)
        ym0 = scr_p.tile([H, 32], F32, name="ym0", tag="ym0")
        nc.vector.tensor_scalar(ym0[:, 0:1], ioy[:, 0:1], 2, None, op0=ALU.is_ge)
        ym1 = scr_p.tile([H, 32], F32, name="ym1", tag="ym1")
        nc.vector.tensor_scalar(ym1[:, 0:1], ioy[:, 0:1], 126, None, op0=ALU.is_lt)
        ym = scr_p.tile([H, 32], F32, name="ym", tag="ym")
        nc.vector.tensor_tensor(ym[:, 0:1], ym0[:, 0:1], ym1[:, 0:1], op=ALU.mult)

        valid = field_p.tile([H, FX], BF16, name="valid", tag="valid")
        nc.vector.tensor_tensor(valid[:], gt_b[:], xm0[:], op=ALU.mult)
        nc.vector.tensor_tensor(valid[:], valid[:], xm1[:], op=ALU.mult)
        nc.vector.tensor_scalar(valid[:], valid[:], ym[:, 0:1], None, op0=ALU.mult)

        R = scr_p.tile([128, 32], F32, name="R", tag="R")
        nc.vector.memset(R[:], 0.0)
        nc.vector.tensor_reduce(R[:, 1:2], valid[:], axis=AX.X, op=ALU.add)
        nc.vector.tensor_reduce(R[:, 2:3], gt_b[:], axis=AX.X, op=ALU.add)

        # ---- dot fields ([y, x]) ---------------------------------------
        fields = {}
        for s in [(0, 0)] + SHIFTS:
            f = field_p.tile([H, FX], F32, name=f"dot_{s[0]}_{s[1]}",
                             tag=f"dot_{s[0]}_{s[1]}")
            nc.gpsimd.memset(f[:], 0.0)
            fields[s] = f

        A = field_p.tile([H, FX], F32, name="accA", tag="accA")
        nc.gpsimd.memset(A[:], 0.0)

        # ---- main per-half loop ----------------------------------------
        for h in range(2):
            r0 = HH * h
            nflat = (SLAB_ROWS if h == 0 else HH) * W
            # issue both er-chunk loads before the xodd loads: the first
            # even-dx multiplies need er only, and queueing xodd first
            # delays er-c1 by a full transfer (~17us DVE stall measured)
            er_ch, xo_ch = [], []
            for c in range(2):
                e = er_p.tile([128, L_SLAB], BF16, name=f"er{c}",
                              tag=f"er{c}")
                nc.gpsimd.memset(e[:, nflat:L_SLAB], 0.0)
                nc.gpsimd.dma_start(
                    out=_ap(e, 0, [[L_SLAB, 128], [1, nflat]]),
                    in_=_ap(er_d.ap(), c * 128 * H * W + r0 * W,
                            [[H * W, 128], [1, nflat]]))
                er_ch.append(e)
            for c in range(2):
                x = xo_p.tile([128, L_SLAB], BF16, name=f"xo{c}",
                              tag=f"xo{c}")
                nodd = min(nflat, H * W - r0 * W - 1)
                nc.gpsimd.memset(x[:, nodd:L_SLAB], 0.0)
                nc.gpsimd.dma_start(
                    out=_ap(x, 0, [[L_SLAB, 128], [1, nodd]]),
                    in_=_ap(er_d.ap(), c * 128 * H * W + r0 * W + 1,
                            [[H * W, 128], [1, nodd]]))
                xo_ch.append(x)

            for s in [(0, 0)] + SHIFTS:
                dy, dx = s
                off = dy * W + dx
                prods = []
                for c in range(2):
                    p = prod_p.tile([128, L_RED], BF16, name=f"prod{c}",
                                    tag=f"prod{c}")
                    if dx % 2 == 0:
                        in1 = er_ch[c][:, off:off + L_RED]
                    else:
                        in1 = xo_ch[c][:, off - 1:off - 1 + L_RED]
                    nc.vector.tensor_tensor(
                        p[:], er_ch[c][:, 0:L_RED], in1, op=ALU.mult)
                    prods.append(p)

                # block b -> psum row b (one-hot stationary); the block's
                # 512 pixels are the strided y-rows {b, b+16, b+32, b+48}
                # so the staging tile fans out with canonical DMAs below.
                ps = psum_p.tile([128, 512], F32, name="ps", tag="ps")
                n_mm = 2 * NB
                j = 0
                # descending b: the first matmul (b=NB-1) covers rows
                # [0:NB] and start=True-initializes them; later partial
                # writes accumulate into initialized rows only.
                for b in reversed(range(NB)):
                    for c in range(2):
                        nc.tensor.matmul(
                            ps[0:b + 1, 0:512], sel_view(b),
                            _ap(prods[c], 128 * b,
                                [[L_RED, 128], [128 * NB, 4], [1, W]]),
                            start=(j == 0), stop=(j == n_mm - 1),
                            skip_group_check=True)
                        j += 1

                st = st_p.tile([NB, 512], F32, name="st", tag="st")
                nc.scalar.copy(st[:], ps[0:NB, 0:512])

                # st[g, 128q + x] = dot(y = 16q + g, x): 4 DMAs, each to
                # 16 contiguous field partitions (pure partition dim0)
                f = fields[s]
                for q in range(4):
                    nc.sync.dma_start(
                        out=_ap(f, (r0 + 16 * q) * FX + FOFF,
                                [[FX, NB], [1, W]]),
                        in_=_ap(st, 128 * q, [[512, NB], [1, W]]))

        # ---- rn = 1 / max(sqrt(n2), eps) -------------------------------
        rn1 = scr_p.tile([H, FX], F32, name="rn1", tag="rn1")
        nc.vector.memset(rn1[:], 0.0)
        nc.scalar.activation(rn1[:], fields[(0, 0)][:], ACTF.Sqrt)
        nc.vector.tensor_scalar(rn1[:], rn1[:], 1e-8, None, op0=ALU.max)
        rn = field_p.tile([H, FX], F32, name="rn", tag="rn")
        nc.vector.reciprocal(rn[:], rn1[:])

        # ---- dy-shifted copies (engines can't start at partition k) ----
        # f_dk[y, x] = f[y + k, x]; tail rows zero.
        shifted = {0: {"rn": rn, "segi": segi, "valid": valid}}
        for k in (1, 2):
            sd = {}
            for nm, src in (("rn", rn), ("segi", segi), ("valid", valid)):
                t = field_p.tile([H, FX], src.dtype, name=f"{nm}_d{k}",
                                 tag=f"{nm}_d{k}")
                nc.gpsimd.memset(t[:], 0)
                nc.sync.dma_start(
                    out=_ap(t, 0, [[FX, H - k], [1, FX]]),
                    in_=_ap(src, k * FX, [[FX, H - k], [1, FX]]))
                sd[nm] = t
            shifted[k] = sd

        # ---- pointwise per shift ---------------------------------------
        for s in SHIFTS:
            dy, dx = s
            b_ = np.s_[:, FOFF:FOFF + W]
            sh = np.s_[:, FOFF + dx:FOFF + dx + W]
            rn_s = shifted[dy]["rn"]
            segi_s = shifted[dy]["segi"]
            valid_s = shifted[dy]["valid"]

            lab = scr_p.tile([H, FX], BF16, name="lab", tag="lab")
            nc.vector.tensor_tensor(lab[b_], segi[b_], segi_s[sh],
                                    op=ALU.is_equal)
            Wt = scr_p.tile([H, FX], BF16, name="Wt", tag="Wt")
            nc.vector.tensor_tensor(Wt[b_], valid[b_], valid_s[sh],
                                    op=ALU.add)
            t1 = scr_p.tile([H, FX], F32, name="t1", tag="t1")
            nc.vector.tensor_tensor(t1[b_], fields[s][b_], rn[b_],
                                    op=ALU.mult)
            cosb = scr_p.tile([H, FX], BF16, name="cosb", tag="cosb")
            nc.vector.tensor_tensor(cosb[b_], t1[b_], rn_s[sh], op=ALU.mult)
            d = scr_p.tile([H, FX], BF16, name="d", tag="d")
            nc.vector.tensor_tensor(d[b_], cosb[b_], lab[b_],
                                    op=ALU.subtract)
            e2 = scr_p.tile([H, FX], BF16, name="e2", tag="e2")
            nc.vector.tensor_tensor(e2[b_], d[b_], d[b_], op=ALU.mult)
            fw = scr_p.tile([H, FX], BF16, name="fw", tag="fw")
            nc.vector.tensor_tensor(fw[b_], e2[b_], Wt[b_], op=ALU.mult)
            nc.vector.tensor_tensor(A[b_], A[b_], fw[b_], op=ALU.add)

        # ---- final reduction -------------------------------------------
        nc.vector.tensor_reduce(R[:, 0:1], A[:], axis=AX.X, op=ALU.add)

        ps2 = psum_p.tile([128, 512], F32, name="ps2", tag="ps")
        nc.tensor.matmul(ps2[0:1, 0:4], ones_f[:, 0:1], R[:, 0:4],
                         start=True, stop=True)
        scal = scr_p.tile([1, 32], F32, name="scal", tag="scal")
        nc.scalar.copy(scal[0:1, 0:4], ps2[0:1, 0:4])
        # scal: 0=S, 1=cnt, 2=gtbsum | 4=include, 5=max(cnt,1), 6=1/max, 7=loss
        nc.vector.tensor_scalar(scal[0:1, 4:5], scal[0:1, 2:3], 0.0, None,
                                op0=ALU.is_gt)
        nc.vector.tensor_scalar(scal[0:1, 5:6], scal[0:1, 1:2], 1.0, None,
                                op0=ALU.max)
        nc.vector.reciprocal(scal[0:1, 6:7], scal[0:1, 5:6])
        nc.vector.tensor_tensor(scal[0:1, 7:8], scal[0:1, 0:1],
                                scal[0:1, 6:7], op=ALU.mult)
        nc.vector.tensor_tensor(scal[0:1, 7:8], scal[0:1, 7:8],
                                scal[0:1, 4:5], op=ALU.mult)
        nc.vector.tensor_scalar(scal[0:1, 7:8], scal[0:1, 7:8],
                                1.0 / 24.0, None, op0=ALU.mult)

        outt = scr_p.tile([1, 32], F32, name="outt", tag="outt")
        nc.vector.tensor_copy(outt[0:1, 0:1], scal[0:1, 7:8])
        nc.vector.tensor_copy(outt[0:1, 1:2], scal[0:1, 4:5])
        nc.sync.dma_start(out=out_d.ap(), in_=outt[0:1, 0:2])


_NC_CACHE = {}


def get_nc():
    if "nc" not in _NC_CACHE:
        nc = bacc.Bacc("TRN2", target_bir_lowering=False, debug=False)
        build_kernel(nc)
        _NC_CACHE["nc"] = nc
    return _NC_CACHE["nc"]


def kernel(er_input, seg_label, gt_boundary_seg):
    er = np.ascontiguousarray(np.asarray(er_input, dtype=np.float32))
    seg = np.ascontiguousarray(np.asarray(seg_label, dtype=np.int32))
    gtb = np.ascontiguousarray(np.asarray(gt_boundary_seg, dtype=np.int32))
    assert er.shape == (B, C, H, W), er.shape

    nc = get_nc()
    from concourse.bass_utils import run_bass_kernel_spmd

    in_maps = [
        {"er": er[i], "seg": seg[i], "gtb": gtb[i]} for i in range(B)
    ]
    res = run_bass_kernel_spmd(nc, in_maps, list(range(B)))
    outs = [res.results[i]["out"] for i in range(B)]
    loss_nums = np.array([o[0, 0] for o in outs], dtype=np.float64)
    incs = np.array([o[0, 1] for o in outs], dtype=np.float64)
    loss = loss_nums.sum() / max(incs.sum(), 1.0)
    return np.float32(loss)


# revision 34
# speedup vs baseline: 1.3530x; 1.0078x over previous
"""Trainium2 Bass kernel for nn_CBL_1632087573343 (boundary context loss).

Data-parallel over batch: 8 images -> 8 NeuronCores, one image per core.

Per-core algorithm (reproduces reference._context_loss for one image):
  - er image stored as bf16 [c-chunk(128) x 2, 66*128] flat slabs,
    processed in 2 row-halves; a 1-element-shifted copy (xodd) keeps the
    DVE tensor_tensor multiplies 4B-aligned (2x perf mode) for odd offsets.
  - For each of 12 canonical shifts s (+- pairs folded via weight
    W_s = valid + valid(.+s)) plus the norm pass (s=0):
      DVE: prod_c = er_c * er_c_shifted   (bf16, flat offset dy*128+dx)
      PE:  channel reduction via matmuls with ONE-HOT-COLUMN stationaries:
           block b (512 pixels) uses stationary = window view of a
           [128, 128+NB] tile whose only nonzero column selects output
           partition b; all blocks accumulate into one PSUM [128, 512]
           bank, so 16 blocks x 2 chunks of a half land as rows 0..15.
           Moving operand is the product (N=512 @ 2.4 GHz, LDW hidden).
      ACT: copy psum[0:16, :] -> st [16, 512]
      DMA: fan st out to the dot field tile [y=128, 2|128|2] ([y, x]).
  - Pointwise on [128 y, 132] tiles: dy handled by DMA-shifted copies of
    rn/seg/valid (engines cannot start at partition 1/2), dx by free-dim
    offsets.  cos = dot*rn*rn_s, d = cos - (seg==seg_s), A += d*d*W_s.
  - Reduce A / valid / gt_b; assemble per-image (loss_num, include).
Host combines: loss = sum(loss_num) / max(sum(include), 1).
"""

import sys

sys.path.insert(0, "/opt/trn_rl_repo")

import numpy as np

import concourse.bass as bass
import concourse.tile as tile
from concourse import bacc, mybir

DT = mybir.dt
F32 = DT.float32
BF16 = DT.bfloat16
I32 = DT.int32
ALU = mybir.AluOpType
ACTF = mybir.ActivationFunctionType
AX = mybir.AxisListType

B, C, H, W = 8, 256, 128, 128
HH = 64                          # rows per half
SLAB_ROWS = HH + 2               # rows resident per half (dy<=2 read-ahead)
L_SLAB = 8512                    # >= 66*128+4, padded to a 128B multiple
L_RED = HH * W                   # 8192 columns reduced per (half, shift)
NB = 16                          # 512-pixel blocks per (half, shift)
FX = 192                         # field tile free size (128B-aligned): 2 | 128 x | pad
FOFF = 2                         # x offset inside field tiles

# canonical half of the 24-shift set; even-dx first so the odd-dx slab copy
# (single-buffered) can load while even shifts compute
SHIFTS = [(0, 2), (1, -2), (1, 0), (1, 2), (2, -2), (2, 0), (2, 2),
          (0, 1), (1, -1), (1, 1), (2, -1), (2, 1)]


def _ap(t, offset, dims):
    return bass.AP(t.tensor, offset, [list(d) for d in dims])


def build_kernel(nc):
    er_d = nc.dram_tensor("er", [C, H, W], F32, kind="ExternalInput")
    seg_d = nc.dram_tensor("seg", [H, W], I32, kind="ExternalInput")
    gtb_d = nc.dram_tensor("gtb", [H, W], I32, kind="ExternalInput")
    out_d = nc.dram_tensor("out", [1, 2], F32, kind="ExternalOutput")

    with tile.TileContext(nc) as tc:
        _build(tc, er_d, seg_d, gtb_d, out_d)
    nc.compile()
    return nc


def _build(tc, er_d, seg_d, gtb_d, out_d):
    nc = tc.nc
    from contextlib import ExitStack

    with ExitStack() as ctx:
        const_p = ctx.enter_context(tc.tile_pool(name="const", bufs=1))
        er_p = ctx.enter_context(tc.tile_pool(name="erp", bufs=2))
        xo_p = ctx.enter_context(tc.tile_pool(name="xop", bufs=1))
        prod_p = ctx.enter_context(tc.tile_pool(name="prodp", bufs=2))
        field_p = ctx.enter_context(tc.tile_pool(name="fieldp", bufs=1))
        st_p = ctx.enter_context(tc.tile_pool(name="stp", bufs=3))
        scr_p = ctx.enter_context(tc.tile_pool(name="scrp", bufs=1))
        psum_p = ctx.enter_context(
            tc.tile_pool(name="psump", bufs=3, space="PSUM"))

        ones_f = const_p.tile([128, 32], F32, name="ones_f", tag="ones_f")
        nc.vector.memset(ones_f[:], 1.0)
        # one-hot column bank: sel[:, 128+NB-1-b : 256+NB-1-b] has its only
        # nonzero (ones) column at position b
        SELW = 320
        sel = const_p.tile([128, SELW], BF16, name="sel", tag="sel")
        nc.gpsimd.memset(sel[:], 0.0)
        nc.vector.memset(sel[:, 128 + NB - 1:128 + NB], 1.0)

        P0 = 128 + NB - 1   # absolute position of the ones column

        def sel_view(b):
            # b+1 columns ending at the ones column: output rows 0..b,
            # row b = column sums. Short stationary keeps LDWEIGHTS tiny.
            return sel[:, P0 - b:P0 + 1]

        # ---- label fields ([y, x] layout) ------------------------------
        segi = field_p.tile([H, FX], I32, name="segi", tag="segi")
        nc.gpsimd.memset(segi[:], 0)
        nc.sync.dma_start(out=segi[:, FOFF:FOFF + W], in_=seg_d.ap())
        gtbi = field_p.tile([H, FX], I32, name="gtbi", tag="gtbi")
        nc.gpsimd.memset(gtbi[:], 0)
        nc.sync.dma_start(out=gtbi[:, FOFF:FOFF + W], in_=gtb_d.ap())

        segb = scr_p.tile([H, FX], BF16, name="segb", tag="segb")
        nc.vector.tensor_copy(segb[:], segi[:])
        gtbb = scr_p.tile([H, FX], BF16, name="gtbb", tag="gtbb")
        nc.vector.tensor_copy(gtbb[:], gtbi[:])
        gt_b = field_p.tile([H, FX], BF16, name="gt_b", tag="gt_b")
        nc.vector.tensor_tensor(gt_b[:], segb[:], gtbb[:], op=ALU.mult)

        # interior: x (free col) in [FOFF+2, FOFF+126), y (part) in [2,126)
        iox = scr_p.tile([H, FX], I32, name="iox", tag="iox")
        nc.gpsimd.iota(iox[:], [[1, FX]], channel_multiplier=0)
        xm0 = scr_p.tile([H, FX], BF16, name="xm0", tag="xm0")
        nc.vector.tensor_scalar(xm0[:], iox[:], FOFF + 2, None, op0=ALU.is_ge)
        xm1 = scr_p.tile([H, FX], BF16, name="xm1", tag="xm1")
        nc.vector.tensor_scalar(xm1[:], iox[:], FOFF + 126, None,
                                op0=ALU.is_lt)
        ioy = scr_p.tile([H, 32], I32, name="ioy", tag="ioy")
        nc.gpsimd.iota(ioy[:, 0:1], [[1, 1]], channel_multiplier=1)
        ym0 = scr_p.tile([H, 32], F32, name="ym0", tag="ym0")
        nc.vector.tensor_scalar(ym0[:, 0:1], ioy[:, 0:1], 2, None, op0=ALU.is_ge)
        ym1 = scr_p.tile([H, 32], F32, name="ym1", tag="ym1")
        nc.vector.tensor_scalar(ym1[:, 0:1], ioy[:, 0:1], 126, None, op0=ALU.is_lt)
        ym = scr_p.tile([H, 32], F32, name="ym", tag="ym")
        nc.vector.tensor_tensor(ym[:, 0:1], ym0[:, 0:1], ym1[:, 0:1], op=ALU.mult)

        valid = field_p.tile([H, FX], BF16, name="valid", tag="valid")
        nc.vector.tensor_tensor(valid[:], gt_b[:], xm0[:], op=ALU.mult)
        nc.vector.tensor_tensor(valid[:], valid[:], xm1[:], op=ALU.mult)
        nc.vector.tensor_scalar(valid[:], valid[:], ym[:, 0:1], None, op0=ALU.mult)

        R = scr_p.tile([128, 32], F32, name="R", tag="R")
        nc.vector.memset(R[:], 0.0)
        nc.vector.tensor_reduce(R[:, 1:2], valid[:], axis=AX.X, op=ALU.add)
        nc.vector.tensor_reduce(R[:, 2:3], gt_b[:], axis=AX.X, op=ALU.add)

        # ---- dot fields ([y, x]) ---------------------------------------
        fields = {}
        for s in [(0, 0)] + SHIFTS:
            f = field_p.tile([H, FX], F32, name=f"dot_{s[0]}_{s[1]}",
                             tag=f"dot_{s[0]}_{s[1]}")
            nc.gpsimd.memset(f[:], 0.0)
            fields[s] = f

        A = field_p.tile([H, FX], F32, name="accA", tag="accA")
        nc.gpsimd.memset(A[:], 0.0)

        # ---- main per-half loop ----------------------------------------
        for h in range(2):
            r0 = HH * h
            nflat = (SLAB_ROWS if h == 0 else HH) * W
            # issue both er-chunk loads before the xodd loads: the first
            # even-dx multiplies need er only, and queueing xodd first
            # delays er-c1 by a full transfer (~17us DVE stall measured)
            er_ch, xo_ch = [], []
            for c in range(2):
                e = er_p.tile([128, L_SLAB], BF16, name=f"er{c}",
                              tag=f"er{c}")
                nc.gpsimd.memset(e[:, nflat:L_SLAB], 0.0)
                nc.gpsimd.dma_start(
                    out=_ap(e, 0, [[L_SLAB, 128], [1, nflat]]),
                    in_=_ap(er_d.ap(), c * 128 * H * W + r0 * W,
                            [[H * W, 128], [1, nflat]]))
                er_ch.append(e)
            for c in range(2):
                x = xo_p.tile([128, L_SLAB], BF16, name=f"xo{c}",
                              tag=f"xo{c}")
                nodd = min(nflat, H * W - r0 * W - 1)
                nc.gpsimd.memset(x[:, nodd:L_SLAB], 0.0)
                nc.gpsimd.dma_start(
                    out=_ap(x, 0, [[L_SLAB, 128], [1, nodd]]),
                    in_=_ap(er_d.ap(), c * 128 * H * W + r0 * W + 1,
                            [[H * W, 128], [1, nodd]]))
                xo_ch.append(x)

            for s in [(0, 0)] + SHIFTS:
                dy, dx = s
                off = dy * W + dx
                prods = []
                for c in range(2):
                    p = prod_p.tile([128, L_RED], BF16, name=f"prod{c}",
                                    tag=f"prod{c}")
                    if dx % 2 == 0:
                        in1 = er_ch[c][:, off:off + L_RED]
                    else:
                        in1 = xo_ch[c][:, off - 1:off - 1 + L_RED]
                    nc.vector.tensor_tensor(
                        p[:], er_ch[c][:, 0:L_RED], in1, op=ALU.mult)
                    prods.append(p)

                # block b -> psum row b (one-hot stationary); the block's
                # 512 pixels are the strided y-rows {b, b+16, b+32, b+48}
                # so the staging tile fans out with canonical DMAs below.
                ps = psum_p.tile([128, 512], F32, name="ps", tag="ps")
                n_mm = 2 * NB
                j = 0
                # descending b: the first matmul (b=NB-1) covers rows
                # [0:NB] and start=True-initializes them; later partial
                # writes accumulate into initialized rows only.
                for b in reversed(range(NB)):
                    for c in range(2):
                        nc.tensor.matmul(
                            ps[0:b + 1, 0:512], sel_view(b),
                            _ap(prods[c], 128 * b,
                                [[L_RED, 128], [128 * NB, 4], [1, W]]),
                            start=(j == 0), stop=(j == n_mm - 1),
                            skip_group_check=True)
                        j += 1

                st = st_p.tile([NB, 512], F32, name="st", tag="st")
                nc.scalar.copy(st[:], ps[0:NB, 0:512])

                # st[g, 128q + x] = dot(y = 16q + g, x): 4 DMAs, each to
                # 16 contiguous field partitions (pure partition dim0)
                f = fields[s]
                for q in range(4):
                    nc.sync.dma_start(
                        out=_ap(f, (r0 + 16 * q) * FX + FOFF,
                                [[FX, NB], [1, W]]),
                        in_=_ap(st, 128 * q, [[512, NB], [1, W]]))

        # ---- rn = 1 / max(sqrt(n2), eps) -------------------------------
        rn1 = scr_p.tile([H, FX], F32, name="rn1", tag="rn1")
        nc.vector.memset(rn1[:], 0.0)
        nc.scalar.activation(rn1[:], fields[(0, 0)][:], ACTF.Sqrt)
        nc.vector.tensor_scalar(rn1[:], rn1[:], 1e-8, None, op0=ALU.max)
        rn = field_p.tile([H, FX], F32, name="rn", tag="rn")
        nc.vector.reciprocal(rn[:], rn1[:])

        # ---- dy-shifted copies (engines can't start at partition k) ----
        # f_dk[y, x] = f[y + k, x]; tail rows zero.
        shifted = {0: {"rn": rn, "segi": segi, "valid": valid}}
        for k in (1, 2):
            sd = {}
            for nm, src in (("rn", rn), ("segi", segi), ("valid", valid)):
                t = field_p.tile([H, FX], src.dtype, name=f"{nm}_d{k}",
                                 tag=f"{nm}_d{k}")
                nc.gpsimd.memset(t[:], 0)
                nc.sync.dma_start(
                    out=_ap(t, 0, [[FX, H - k], [1, FX]]),
                    in_=_ap(src, k * FX, [[FX, H - k], [1, FX]]))
                sd[nm] = t
            shifted[k] = sd

        # ---- pointwise per shift ---------------------------------------
        for s in SHIFTS:
            dy, dx = s
            b_ = np.s_[:, FOFF:FOFF + W]
            sh = np.s_[:, FOFF + dx:FOFF + dx + W]
            rn_s = shifted[dy]["rn"]
            segi_s = shifted[dy]["segi"]
            valid_s = shifted[dy]["valid"]

            lab = scr_p.tile([H, FX], BF16, name="lab", tag="lab")
            nc.vector.tensor_tensor(lab[b_], segi[b_], segi_s[sh],
                                    op=ALU.is_equal)
            Wt = scr_p.tile([H, FX], BF16, name="Wt", tag="Wt")
            nc.vector.tensor_tensor(Wt[b_], valid[b_], valid_s[sh],
                                    op=ALU.add)
            t1 = scr_p.tile([H, FX], F32, name="t1", tag="t1")
            nc.vector.tensor_tensor(t1[b_], fields[s][b_], rn[b_],
                                    op=ALU.mult)
            cosb = scr_p.tile([H, FX], BF16, name="cosb", tag="cosb")
            nc.vector.tensor_tensor(cosb[b_], t1[b_], rn_s[sh], op=ALU.mult)
            d = scr_p.tile([H, FX], BF16, name="d", tag="d")
            nc.vector.tensor_tensor(d[b_], cosb[b_], lab[b_],
                                    op=ALU.subtract)
            e2 = scr_p.tile([H, FX], BF16, name="e2", tag="e2")
            nc.vector.tensor_tensor(e2[b_], d[b_], d[b_], op=ALU.mult)
            fw = scr_p.tile([H, FX], BF16, name="fw", tag="fw")
            nc.vector.tensor_tensor(fw[b_], e2[b_], Wt[b_], op=ALU.mult)
            nc.vector.tensor_tensor(A[b_], A[b_], fw[b_], op=ALU.add)

        # ---- final reduction -------------------------------------------
        nc.vector.tensor_reduce(R[:, 0:1], A[:], axis=AX.X, op=ALU.add)

        ps2 = psum_p.tile([128, 512], F32, name="ps2", tag="ps")
        nc.tensor.matmul(ps2[0:1, 0:4], ones_f[:, 0:1], R[:, 0:4],
                         start=True, stop=True)
        scal = scr_p.tile([1, 32], F32, name="scal", tag="scal")
        nc.scalar.copy(scal[0:1, 0:4], ps2[0:1, 0:4])
        # scal: 0=S, 1=cnt, 2=gtbsum | 4=include, 5=max(cnt,1), 6=1/max, 7=loss
        nc.vector.tensor_scalar(scal[0:1, 4:5], scal[0:1, 2:3], 0.0, None,
                                op0=ALU.is_gt)
        nc.vector.tensor_scalar(scal[0:1, 5:6], scal[0:1, 1:2], 1.0, None,
                                op0=ALU.max)
        nc.vector.reciprocal(scal[0:1, 6:7], scal[0:1, 5:6])
        nc.vector.tensor_tensor(scal[0:1, 7:8], scal[0:1, 0:1],
                                scal[0:1, 6:7], op=ALU.mult)
        nc.vector.tensor_tensor(scal[0:1, 7:8], scal[0:1, 7:8],
                                scal[0:1, 4:5], op=ALU.mult)
        nc.vector.tensor_scalar(scal[0:1, 7:8], scal[0:1, 7:8],
                                1.0 / 24.0, None, op0=ALU.mult)

        outt = scr_p.tile([1, 32], F32, name="outt", tag="outt")
        nc.vector.tensor_copy(outt[0:1, 0:1], scal[0:1, 7:8])
        nc.vector.tensor_copy(outt[0:1, 1:2], scal[0:1, 4:5])
        nc.sync.dma_start(out=out_d.ap(), in_=outt[0:1, 0:2])


_NC_CACHE = {}


def get_nc():
    if "nc" not in _NC_CACHE:
        nc = bacc.Bacc("TRN2", target_bir_lowering=False, debug=False)
        build_kernel(nc)
        _NC_CACHE["nc"] = nc
    return _NC_CACHE["nc"]


def kernel(er_input, seg_label, gt_boundary_seg):
    er = np.ascontiguousarray(np.asarray(er_input, dtype=np.float32))
    seg = np.ascontiguousarray(np.asarray(seg_label, dtype=np.int32))
    gtb = np.ascontiguousarray(np.asarray(gt_boundary_seg, dtype=np.int32))
    assert er.shape == (B, C, H, W), er.shape

    nc = get_nc()
    from concourse.bass_utils import run_bass_kernel_spmd

    in_maps = [
        {"er": er[i], "seg": seg[i], "gtb": gtb[i]} for i in range(B)
    ]
    res = run_bass_kernel_spmd(nc, in_maps, list(range(B)))
    outs = [res.results[i]["out"] for i in range(B)]
    loss_nums = np.array([o[0, 0] for o in outs], dtype=np.float64)
    incs = np.array([o[0, 1] for o in outs], dtype=np.float64)
    loss = loss_nums.sum() / max(incs.sum(), 1.0)
    return np.float32(loss)
